# revision 1
# baseline (speedup 1.0000x reference)
"""Trainium2 Bass kernel for nn_D2GroupConvolutionLayer (D2-equivariant GAT).

Math: for each output view g and input view h, the layer computes a GAT with a
GLOBAL softmax over edges (not per-destination).  Because the edge score
factorizes as score(e) = u[src(e)] + v[dst(e)], the whole
gather -> softmax -> scatter-add pipeline collapses to dense algebra:

    out_gh = diag(b) . M . diag(a) . H / (b^T M a)

where a = exp(u - max u), b = exp(v - max v) are per-node scalars and
M[d, s] = multiplicity of edge s->d (self-loops included) is a FIXED 0/1/2
integer matrix that depends only on edge_index.  M is built on the host (pure
index bookkeeping) and shipped as bf16 (exact for small integers); the device
does only dense matmuls + elementwise work.  No gather/scatter on device.

Sharding: data-parallel over the 8 (batch b, output view g) pairs, one
NeuronCore each; all-to-nothing communication.
"""

import os
import sys
from contextlib import ExitStack

for _p in ("/opt/trn_rl_repo/concourse", "/opt/trn_rl_repo"):
    if _p not in sys.path:
        sys.path.insert(0, _p)

import ml_dtypes  # noqa: E402
import numpy as np  # noqa: E402

import concourse.bass as bass  # noqa: E402
import concourse.bacc as bacc  # noqa: E402
import concourse.mybir as mybir  # noqa: E402
import concourse.tile as tile  # noqa: E402
import concourse.tile_utils as tile_utils  # noqa: E402
import bass_rust  # noqa: E402

# Problem constants (hardcoded per harness contract).
B, V, N, F, O = 2, 4, 2048, 128, 512
NT = N // 128  # node tiles
NEG_SLOPE = 0.2
F32, F32R, BF16 = mybir.dt.float32, mybir.dt.float32r, mybir.dt.bfloat16

# Stock cap leaves 16KB/partition unused on trn2 (224 phys / 208 usable).
tile_utils.max_sbuf_usage = 204 * 1024


class _TileContext(tile.TileContext):
    """Splits the exit-drain's sem waits across single-wait carrier nops.

    Walrus caps sync waits at 1/instruction (2 for EventSemaphore); the stock
    _drain_and_barrier attaches every outstanding DMA/engine sem wait to one
    Drain and fails codegen with "Too many sync wait commands".
    """

    def _drain_and_barrier(self, tick_clock, wait_clock):
        nc = self.nc
        probe = nc.sync.nop(nofuse=True)
        wait_clock.add_sem_waits(
            probe.ins, bass_rust.ScopedClock({None: tick_clock.global_clock})
        )
        si = probe.ins.sync_info
        if si is not None and si.on_wait and len(si.on_wait) > 1:
            waits = list(si.on_wait)
            si.on_wait = [waits[0]]
            for w in waits[1:]:
                carrier = nc.sync.nop(nofuse=True)
                carrier.ins.sync_info = mybir.SyncInfo(on_wait=[w], on_update=[])
        nc.sync.drain()
        nc.all_engine_barrier()
        popped = nc._tile_sem_poison_stack.pop()
        assert popped is self._sem_poison
        nc.clear_and_free_semaphores(list(self.sems.allocated().values()))
        nc.all_engine_barrier()


def _build_program():
    nc = bacc.Bacc("TRN2", target_bir_lowering=False, debug=False)

    OA = O + 1  # Haug gets a 513th column equal to a, so G's last column is M@a

    xpair_d = nc.dram_tensor("xpair", [V, 2, 128, N], F32R, kind="ExternalInput").ap()
    wsel_d = nc.dram_tensor("wsel", [V, 2, 128, O], F32R, kind="ExternalInput").ap()
    mt_d = nc.dram_tensor("mt", [NT, 128, N], BF16, kind="ExternalInput").ap()
    attb_d = nc.dram_tensor("attb", [128, 2 * O], BF16, kind="ExternalInput").ap()
    biasb_d = nc.dram_tensor("biasb", [128, O], F32, kind="ExternalInput").ap()
    out_d = nc.dram_tensor("out", [NT, 128, O], F32, kind="ExternalOutput").ap()

    with ExitStack() as ctx:
        tc = ctx.enter_context(tile.TileContext(nc))
        pool = ctx.enter_context(tc.tile_pool(name="main", bufs=1))
        xpool = ctx.enter_context(tc.tile_pool(name="x", bufs=10))
        hpool = ctx.enter_context(tc.tile_pool(name="hg", bufs=2))
        lpool = ctx.enter_context(tc.tile_pool(name="l", bufs=3))
        spool = ctx.enter_context(tc.tile_pool(name="s", bufs=2))
        stpool = ctx.enter_context(tc.tile_pool(name="st", bufs=2))
        pp = ctx.enter_context(tc.tile_pool(name="ps", bufs=8, space="PSUM"))

        # ---- persistent SBUF tensors ----
        attb = pool.tile([128, 2 * O], BF16)
        biasb = pool.tile([128, O], F32)
        wsel = pool.tile([128, V, 2, O], F32R)
        mt = pool.tile([128, NT, N], BF16)
        out_acc = pool.tile([128, NT, O], F32)
        tmp = pool.tile([128, NT, O], F32)  # b * G staging, pre-1/z
        mrow = pool.tile([1, 128], F32)
        ones = pool.tile([128, 1], F32)
        ones_row = pool.tile([1, 128], F32)
        zp = pool.tile([128, V], F32)
        z1 = pool.tile([1, V], F32)

        nc.sync.dma_start(attb[:], attb_d[:])
        nc.sync.dma_start(biasb[:], biasb_d[:])

        nc.vector.memset(ones[:], 1.0)
        nc.vector.memset(ones_row[:], 1.0)

        st = {}

        def h_mms(h):
            """H = x-pair @ W-pair into psum; ACT-copy to Haug bf16."""
            for i in range(2):
                nc.sync.dma_start(wsel[:, h, i, :], wsel_d[h, i])
            # 512-column chunks so the first matmuls start before the whole
            # 1MB view transfer lands
            xpc = []
            for i in range(2):
                row = []
                for c in range(4):
                    xc = xpool.tile([128, 512], F32R, tag="xp", name=f"xp{h}_{i}_{c}")
                    nc.sync.dma_start(
                        xc[:], xpair_d[h, i, :, c * 512 : (c + 1) * 512]
                    )
                    row.append(xc)
                xpc.append(row)
            haug = hpool.tile([128, NT, OA], BF16, tag="haug", name=f"haug{h}")
            u_all = stpool.tile([128, NT], F32, tag="u", name=f"u{h}")
            v_all = stpool.tile([128, NT], F32, tag="v", name=f"v{h}")
            for t in range(NT):
                ph = pp.tile([128, O], F32, tag="ps", name=f"ph{h}_{t}")
                c, col = t // 4, (t % 4) * 128
                nc.tensor.matmul(
                    ph[:], xpc[0][c][:, col : col + 128], wsel[:, h, 0, :],
                    start=True, stop=False,
                )
                nc.tensor.matmul(
                    ph[:], xpc[1][c][:, col : col + 128], wsel[:, h, 1, :],
                    start=False, stop=True,
                )
                nc.scalar.copy(haug[:, t, :O], ph[:])  # psum -> sbuf, bf16
            st[h] = [haug, u_all, v_all]

        def dots_t(h, t):
            """lrelu + att dot-products for node tile t of view h (DVE)."""
            haug, u_all, v_all = st[h][:3]
            hb = haug[:, t, :O]
            lt = lpool.tile([128, O], BF16, tag="l", name=f"lt{h}_{t}")
            nc.vector.scalar_tensor_tensor(
                lt[:], hb, NEG_SLOPE, hb,
                op0=mybir.AluOpType.mult, op1=mybir.AluOpType.max,
            )
            scr = spool.tile([128, O], BF16, tag="s", name=f"scru{h}_{t}")
            nc.vector.scalar_tensor_tensor(
                scr[:], lt[:], 1.0, attb[:, :O],
                op0=mybir.AluOpType.mult, op1=mybir.AluOpType.mult,
                accum_out=u_all[:, t : t + 1],
            )
            scr2 = spool.tile([128, O], BF16, tag="s", name=f"scrv{h}_{t}")
            nc.vector.scalar_tensor_tensor(
                scr2[:], lt[:], 1.0, attb[:, O:],
                op0=mybir.AluOpType.mult, op1=mybir.AluOpType.mult,
                accum_out=v_all[:, t : t + 1],
            )

        def stats(h):
            """Global max; a = exp(u-mu) bf16; b = exp(v-mv); Haug *= a;
            Haug[:, :, 512] = a."""
            haug, u_all, v_all = st[h][:3]
            mstat = stpool.tile([128, 2], F32, tag="mst", name=f"mst{h}")
            m1n = stpool.tile([1, 2], F32, tag="m1n", name=f"m1n{h}")
            negm = stpool.tile([128, 2], F32, tag="negm", name=f"negm{h}")
            for j, stat in ((0, u_all), (1, v_all)):
                nc.vector.reduce_max(
                    mstat[:, j : j + 1], stat[:], axis=mybir.AxisListType.X
                )
                nc.sync.dma_start(mrow[0:1, :], mstat[:, j : j + 1])
                nc.vector.tensor_reduce(
                    m1n[0:1, j : j + 1], mrow[0:1, :],
                    axis=mybir.AxisListType.X, op=mybir.AluOpType.max,
                )
            nc.scalar.mul(m1n[0:1, :], m1n[0:1, :], -1.0)
            pb = pp.tile([128, 2], F32, tag="ps", name=f"pbm{h}")
            nc.tensor.matmul(pb[:], ones_row[:], m1n[:], start=True, stop=True)
            nc.vector.tensor_copy(negm[:], pb[:])
            a_bf = stpool.tile([128, NT], BF16, tag="abf", name=f"abf{h}")
            a_st = stpool.tile([128, NT], F32, tag="ast", name=f"ast{h}")
            b_st = stpool.tile([128, NT], F32, tag="bst", name=f"bst{h}")
            nc.scalar.activation(
                a_bf[:], u_all[:],
                mybir.ActivationFunctionType.Exp, bias=negm[:, 0:1],
            )
            nc.vector.tensor_copy(a_st[:], a_bf[:])
            nc.scalar.activation(
                b_st[:], v_all[:],
                mybir.ActivationFunctionType.Exp, bias=negm[:, 1:2],
            )
            for t in range(NT):
                if t % 2 == 0:
                    nc.scalar.mul(haug[:, t, :O], haug[:, t, :O], a_st[:, t : t + 1])
                else:
                    nc.vector.tensor_scalar(
                        haug[:, t, :O], haug[:, t, :O], a_st[:, t : t + 1],
                        None, op0=mybir.AluOpType.mult,
                    )
            nc.vector.tensor_copy(haug[:, :, O], a_bf[:])  # 513th col = a
            st[h].extend([a_bf, b_st])

        def g_pass(h, hn):
            """G = M @ Haug as N=256 + N=257 matmuls per d-tile; stage
            b*G into tmp; col 512 of the B-half is M@a -> ma_h. Interleaves
            the next view's DVE dots per d-tile."""
            haug, _, _, a_bf, b_st = st[h]
            ma_h = stpool.tile([128, NT], F32, tag="ma", name=f"ma{h}")
            HALF = O // 2
            for d in range(NT):
                pga = pp.tile([128, HALF], F32, tag="ps", name=f"pga{h}_{d}")
                pgb = pp.tile([128, HALF + 1], F32, tag="ps", name=f"pgb{h}_{d}")
                for s in range(NT):
                    lhsT = mt[:, s, bass.ts(d, 128)]
                    nc.tensor.matmul(
                        pga[:], lhsT, haug[:, s, :HALF],
                        start=(s == 0), stop=(s == NT - 1),
                    )
                    nc.tensor.matmul(
                        pgb[:], lhsT, haug[:, s, HALF:OA],
                        start=(s == 0), stop=(s == NT - 1),
                    )
                nc.scalar.mul(tmp[:, d, :HALF], pga[:], b_st[:, d : d + 1])
                nc.scalar.mul(tmp[:, d, HALF:], pgb[:, :HALF], b_st[:, d : d + 1])
                nc.vector.tensor_copy(ma_h[:, d : d + 1], pgb[:, HALF : HALF + 1])
                if hn is not None:
                    dots_t(hn, d)
            st[h].append(ma_h)

        def z_chain(h):
            _, _, _, _, b_st, ma_h = st[h]
            zscr = stpool.tile([128, NT], F32, tag="zscr", name=f"zscr{h}")
            nc.vector.scalar_tensor_tensor(
                zscr[:], ma_h[:], 1.0, b_st[:],
                op0=mybir.AluOpType.mult, op1=mybir.AluOpType.mult,
                accum_out=zp[:, h : h + 1],
            )
            pz = pp.tile([1, 1], F32, tag="ps", name=f"pz{h}")
            nc.tensor.matmul(
                pz[:], ones[:], zp[:, h : h + 1], start=True, stop=True
            )
            nc.vector.reciprocal(z1[0:1, h : h + 1], pz[:])
            nc.vector.tensor_scalar(
                z1[0:1, h : h + 1], z1[0:1, h : h + 1], 1.0 / V, None,
                op0=mybir.AluOpType.mult,
            )
            przb = pp.tile([128, 1], F32, tag="ps", name=f"przb{h}")
            nc.tensor.matmul(
                przb[:], ones_row[:], z1[0:1, h : h + 1], start=True, stop=True
            )
            rzh = stpool.tile([128, 1], F32, tag="rz", name=f"rz{h}")
            nc.vector.tensor_copy(rzh[:], przb[:])
            st[h].append(rzh)

        def scale_pass(h):
            rzh = st[h][6]
            for d in range(NT):
                nc.vector.scalar_tensor_tensor(
                    out_acc[:, d, :], tmp[:, d, :], rzh[:, 0:1],
                    biasb[:] if h == 0 else out_acc[:, d, :],
                    op0=mybir.AluOpType.mult, op1=mybir.AluOpType.add,
                )
                if h == V - 1:
                    nc.sync.dma_start(out_d[d], out_acc[:, d, :])

        # software pipeline over views h
        h_mms(0)
        for t in range(NT):
            dots_t(0, t)
        h_mms(1)
        # mt is first needed by g_pass(0); emitting its 8MB transfer after
        # h1's xp/wsel keeps the first two views' inputs ahead on the rings
        for s in range(NT):
            nc.sync.dma_start(mt[:, s, :], mt_d[s])
        stats(0)
        for h in range(V):
            if 2 <= h + 1 < V:
                h_mms(h + 1)
            g_pass(h, h + 1 if h + 1 < V else None)
            z_chain(h)
            if h + 1 < V:
                stats(h + 1)
            scale_pass(h)

    nc.compile()
    _dedup_ldweights(nc)
    return nc


def _dedup_ldweights(nc):
    """Drop an InstLdweights that reloads the exact weights AP already loaded
    by the previous InstLdweights with no intervening PE instruction that
    could clobber the array (the split-G matmul pairs share one mt tile).
    Cuts the exposed weight-load time of the G pass in half on hardware."""
    pe = mybir.EngineType.PE
    removed = 0
    for bb in nc.m.functions[0].blocks:
        insts = list(bb.instructions)
        out = []
        last_key = None
        for i in insts:
            ty = type(i).__name__
            if ty == "InstLdweights":
                ap = i.ins[0]
                key = (str(ap.memref), ap.offset, str(ap.ap))
                si = i.sync_info
                clean = si is None or (not si.on_wait and not si.on_update)
                if key == last_key and clean:
                    removed += 1
                    continue
                last_key = key
            elif getattr(i, "engine", None) == pe:
                if ty == "InstMatmult":
                    try:
                        ap = i.ins[1]
                        mk = (str(ap.memref), ap.offset, str(ap.ap))
                    except Exception:
                        mk = None
                    if mk != last_key:
                        last_key = None  # self-loading (f32r) or foreign weights
                else:
                    last_key = None
            out.append(i)
        if removed:
            bb.instructions = out
    return removed


_SIGNS = None


def _signs():
    global _SIGNS
    if _SIGNS is None:
        s = np.ones((4, F), dtype=np.float32)
        for r in range(4):
            if r & 1:
                s[r, [0, 2]] = -1.0
            if r & 2:
                s[r, [1, 3]] = -1.0
        _SIGNS = s
    return _SIGNS


def _host_prep(x, edge_index, W, att, bias):
    """Pure relayout/index preprocessing; no float math on tensor data
    beyond sign flips of W rows (exact +-1 scaling)."""
    signs = _signs()
    x = np.ascontiguousarray(x, dtype=np.float32)
    W = np.asarray(W, dtype=np.float32)
    att = np.asarray(att, dtype=np.float32)
    bias = np.asarray(bias, dtype=np.float32)
    ei = np.asarray(edge_index)

    # M^T tiles: mt[s_tile][p, d] = M[d, s_tile*128 + p]
    M = np.zeros((N, N), dtype=np.float32)
    np.add.at(M, (ei[1], ei[0]), 1.0)
    M[np.arange(N), np.arange(N)] += 1.0
    MT = np.ascontiguousarray(M.T)
    mt_tiles = np.ascontiguousarray(
        MT.reshape(NT, 128, N).astype(ml_dtypes.bfloat16)
    )

    W1, W2 = W[:F], W[F:]
    attb = np.ascontiguousarray(
        np.broadcast_to(att.reshape(1, 2 * O), (128, 2 * O))
    ).astype(ml_dtypes.bfloat16)
    biasb = np.ascontiguousarray(np.broadcast_to(bias, (128, O)))

    xT = np.ascontiguousarray(x.transpose(0, 1, 3, 2))  # [B, V, F, N]

    in_maps = []
    for core in range(8):
        b, g = divmod(core, V)
        xpair = np.empty((V, 2, 128, N), dtype=np.float32)
        wselc = np.empty((V, 2, 128, O), dtype=np.float32)
        for h in range(V):
            xpair[h, 0] = xT[b, h]
            xpair[h, 1] = xT[b, g ^ h]
            wselc[h, 0] = signs[h ^ g][:, None] * W1
            wselc[h, 1] = signs[h][:, None] * W2
        in_maps.append(
            {
                "xpair": xpair,
                "wsel": wselc,
                "mt": mt_tiles,
                "attb": attb,
                "biasb": biasb,
            }
        )
    return in_maps


_NC = None


def kernel(x, edge_index, W, att, bias):
    global _NC
    if _NC is None:
        _NC = _build_program()
    in_maps = _host_prep(x, edge_index, W, att, bias)

    from concourse.bass_utils import run_bass_kernel_spmd

    res = run_bass_kernel_spmd(_NC, in_maps, list(range(8)))
    out = np.empty((B, V, N, O), dtype=np.float32)
    for core in range(8):
        b, g = divmod(core, V)
        out[b, g] = res.results[core]["out"].reshape(N, O)
    return out



# revision 35
# speedup vs baseline: 1.5377x; 1.5377x over previous
"""Trainium2 Bass kernel for nn_D2GroupConvolutionLayer (D2-equivariant GAT).

Math: per (output view g, input view h) the layer is a GAT with a GLOBAL
softmax over edges.  score(e) = u[src] + v[dst] factorizes, so
gather -> softmax -> scatter collapses to dense algebra:

    out_gh = diag(b) . M . (a * H) / (b^T M a)

a = exp(u - max u), b = exp(v - max v); M[d, s] = edge multiplicity
(self-loops included), a FIXED small-int matrix -> exact in fp8.

Device design (per core = one (batch, output view) pair):
  - H = xpair @ W: bf16 PE matmuls.  wsel carries 2 extra columns
    W @ att_u, W @ att_v so the linear part of the score dots
    (u_lin = att . H) falls out of the same matmuls for free.
  - lrelu trick: att.lrelu(H) = 0.6 (att.H) + 0.4 (att.|H|); |H| comes from
    one ACT Abs pass, the weighted |H| dots are DVE stt with accum, and the
    0.4 is pre-folded into the att operand on the host.
  - a*H quantized to fp8 e4m3 as hi (haug8) plus residual lo (res8 =
    fp8(q - hi)); G = M @ hi + M @ lo via DoubleRow fp8 matmuls (K=256 per
    instruction, 0.5 cyc/row) recovers bf16-level accuracy at half the
    bf16 PE cost.
  - G-pass for view h runs one pipeline slot AFTER h's elementwise work, so
    the residual chain never stalls the PE.
  - Output: DRAM is preloaded with bias; each view's diag(b)/(Vz)-scaled G
    is accumulated straight into DRAM via gpsimd (SWDGE) accumulate-DMA.
    No out_acc in SBUF, no final copy pass.

Sharding: data-parallel over the 8 (batch b, output view g) pairs, one
NeuronCore each; no cross-core communication.
"""

import sys
from contextlib import ExitStack

for _p in ("/opt/trn_rl_repo/concourse", "/opt/trn_rl_repo"):
    if _p not in sys.path:
        sys.path.insert(0, _p)

import ml_dtypes  # noqa: E402
import numpy as np  # noqa: E402

import concourse.bass as bass  # noqa: E402
import concourse.bacc as bacc  # noqa: E402
import concourse.mybir as mybir  # noqa: E402
import concourse.tile as tile  # noqa: E402
import concourse.tile_utils as tile_utils  # noqa: E402
import bass_rust  # noqa: E402

B, V, N, F, O = 2, 4, 2048, 128, 512
NT = N // 128
NEG_SLOPE = 0.2
F32 = mybir.dt.float32
BF16 = mybir.dt.bfloat16
FP8 = mybir.dt.float8e4
DR = mybir.MatmulPerfMode.DoubleRow
MULT = mybir.AluOpType.mult
ADD = mybir.AluOpType.add
SUB = mybir.AluOpType.subtract
ACTF = mybir.ActivationFunctionType

tile_utils.max_sbuf_usage = 204 * 1024


class _TileContext(tile.TileContext):
    """Splits the exit-drain's sem waits across single-wait carrier nops
    (walrus caps sync waits at 1/instruction)."""

    def _drain_and_barrier(self, tick_clock, wait_clock):
        nc = self.nc
        probe = nc.sync.nop(nofuse=True)
        wait_clock.add_sem_waits(
            probe.ins, bass_rust.ScopedClock({None: tick_clock.global_clock})
        )
        si = probe.ins.sync_info
        if si is not None and si.on_wait and len(si.on_wait) > 1:
            waits = list(si.on_wait)
            si.on_wait = [waits[0]]
            for w in waits[1:]:
                carrier = nc.sync.nop(nofuse=True)
                carrier.ins.sync_info = mybir.SyncInfo(on_wait=[w], on_update=[])
        nc.sync.drain()
        nc.all_engine_barrier()
        popped = nc._tile_sem_poison_stack.pop()
        assert popped is self._sem_poison
        nc.clear_and_free_semaphores(list(self.sems.allocated().values()))
        nc.all_engine_barrier()


def _build_program():
    nc = bacc.Bacc("TRN2", target_bir_lowering=False, debug=False)

    OW = O + 2  # wsel gets u_lin/v_lin columns

    xpair_d = nc.dram_tensor("xpair", [V, 2, 128, N], BF16, kind="ExternalInput").ap()
    wsel_d = nc.dram_tensor("wsel", [V, 2, 128, OW], BF16, kind="ExternalInput").ap()
    mt_d = nc.dram_tensor("mt", [NT, 128, N], FP8, kind="ExternalInput").ap()
    attb_d = nc.dram_tensor("attb04", [128, 2 * O], F32, kind="ExternalInput").ap()
    biasb_d = nc.dram_tensor("biasb", [128, O], F32, kind="ExternalInput").ap()
    out_d = nc.dram_tensor("out", [NT, 128, O], F32, kind="ExternalOutput").ap()

    with ExitStack() as ctx:
        tc = ctx.enter_context(_TileContext(nc))
        pool = ctx.enter_context(tc.tile_pool(name="main", bufs=1))
        xpool = ctx.enter_context(tc.tile_pool(name="x", bufs=16))
        hpool = ctx.enter_context(tc.tile_pool(name="hg", bufs=2))
        gpool = ctx.enter_context(tc.tile_pool(name="grp", bufs=6))
        jpool = ctx.enter_context(tc.tile_pool(name="j", bufs=8))
        tpool = ctx.enter_context(tc.tile_pool(name="tmp", bufs=6))
        stpool = ctx.enter_context(tc.tile_pool(name="st", bufs=4))
        pph = ctx.enter_context(tc.tile_pool(name="ph", bufs=2, space="PSUM"))
        ppg = ctx.enter_context(tc.tile_pool(name="pg", bufs=2, space="PSUM"))
        ppa = ctx.enter_context(tc.tile_pool(name="pa", bufs=1, space="PSUM"))
        ppuv = ctx.enter_context(tc.tile_pool(name="puv", bufs=2, space="PSUM"))
        pps = ctx.enter_context(tc.tile_pool(name="pss", bufs=1, space="PSUM"))

        # ---- persistent SBUF ----
        attb = pool.tile([128, 2 * O], F32)   # 0.4-scaled att, broadcast
        biasb = pool.tile([128, O], F32)
        wsel = pool.tile([128, V, 2, OW], BF16)
        mt = pool.tile([128, NT, N], FP8)
        mrow = pool.tile([1, 2, 128], F32)
        ones = pool.tile([128, 1], F32)
        ones_row = pool.tile([1, 128], F32)
        zp = pool.tile([128, V], F32)
        z1 = pool.tile([1, V], F32)

        nc.vector.memset(ones[:], 1.0)
        nc.vector.memset(ones_row[:], 1.0)

        st = {}

        def x_loads(v):
            xpc = []
            for i in range(2):
                row = []
                for c in range(4):
                    xc = xpool.tile([128, 512], BF16, tag="xp", name=f"xp{v}_{i}_{c}")
                    nc.sync.dma_start(xc[:], xpair_d[v, i, :, c * 512 : (c + 1) * 512])
                    row.append(xc)
                xpc.append(row)
            for i in range(2):
                nc.sync.dma_start(wsel[:, v, i, :], wsel_d[v, i])
            return xpc

        def view_state(v):
            st[v] = dict(
                haug=hpool.tile([128, NT, O], BF16, tag="haug", name=f"haug{v}"),
                haug8=hpool.tile([128, NT, O], FP8, tag="h8", name=f"h8{v}"),
                res8=hpool.tile([128, NT, O], FP8, tag="r8", name=f"r8{v}"),
                uabs=stpool.tile([128, NT], F32, tag="ua", name=f"ua{v}"),
                vabs=stpool.tile([128, NT], F32, tag="va", name=f"va{v}"),
                ulin=stpool.tile([128, NT, 2], F32, tag="ul", name=f"ul{v}"),
            )

        def h_tile(v, t):
            """H matmuls + u/v-lin matmuls + ACT psum drain + dots."""
            s = st[v]
            xpc = s["xpc"]
            ph = pph.tile([128, O], F32, tag="ph", name=f"ph{v}_{t}")
            puv = ppuv.tile([128, 2], F32, tag="puv", name=f"puv{v}_{t}")
            c, col = t // 4, (t % 4) * 128
            for i in range(2):
                nc.tensor.matmul(
                    ph[:], xpc[i][c][:, col : col + 128], wsel[:, v, i, :O],
                    start=(i == 0), stop=(i == 1),
                )
            for i in range(2):
                nc.tensor.matmul(
                    puv[:], xpc[i][c][:, col : col + 128], wsel[:, v, i, O:],
                    start=(i == 0), stop=(i == 1),
                )
            nc.scalar.copy(s["haug"][:, t, :], ph[:])
            nc.vector.tensor_copy(s["ulin"][:, t, :], puv[:])
            if t % 2 == 1:
                g = t // 2
                hrel = gpool.tile([128, 2, O], BF16, tag="hrel", name=f"hrel{v}_{g}")
                nc.vector.tensor_scalar(
                    hrel[:], s["haug"][:, 2 * g : 2 * g + 2, :], 0.0, None,
                    op0=mybir.AluOpType.max,
                )
                s.setdefault("hrel", {})[g] = hrel

        def dots(v, t):
            """u,v relu(H)-dots for one tile (DVE stt with accum)."""
            s = st[v]
            habs = s["hrel"][t // 2]
            ju = jpool.tile([128, O], BF16, tag="jd", name=f"jd{v}_{t}")
            nc.vector.scalar_tensor_tensor(
                ju[:], habs[:, t % 2, :], 1.0, attb[:, :O], op0=MULT, op1=MULT,
                accum_out=s["uabs"][:, t : t + 1],
            )
            jv = jpool.tile([128, O], BF16, tag="jv", name=f"jv{v}_{t}")
            nc.vector.scalar_tensor_tensor(
                jv[:], habs[:, t % 2, :], 1.0, attb[:, O:], op0=MULT, op1=MULT,
                accum_out=s["vabs"][:, t : t + 1],
            )

        def stats(v):
            """u = 0.6 u_lin + sum(0.4 att |H|); global max; a, b = exp."""
            s = st[v]
            u_all = stpool.tile([128, NT], F32, tag="u", name=f"u{v}")
            v_all = stpool.tile([128, NT], F32, tag="v", name=f"v{v}")
            nc.vector.scalar_tensor_tensor(
                u_all[:], s["ulin"][:, :, 0], 0.2, s["uabs"][:], op0=MULT, op1=ADD
            )
            nc.vector.scalar_tensor_tensor(
                v_all[:], s["ulin"][:, :, 1], 0.2, s["vabs"][:], op0=MULT, op1=ADD
            )
            pst = pps.tile([128, O], F32, tag="pst", name=f"pst{v}")
            s["pst"] = pst
            mstat = stpool.tile([128, 2], F32, tag="mst", name=f"mst{v}")
            m1n = stpool.tile([1, 2], F32, tag="m1n", name=f"m1n{v}")
            negm = stpool.tile([128, 2], F32, tag="negm", name=f"negm{v}")
            for j, stat in ((0, u_all), (1, v_all)):
                nc.vector.reduce_max(
                    mstat[:, j : j + 1], stat[:], axis=mybir.AxisListType.X
                )
                nc.sync.dma_start(mrow[0:1, j, :], mstat[:, j : j + 1])
                nc.vector.tensor_reduce(
                    m1n[0:1, j : j + 1], mrow[0:1, j, :],
                    axis=mybir.AxisListType.X, op=mybir.AluOpType.max,
                )
            nc.scalar.mul(m1n[0:1, :], m1n[0:1, :], -1.0)
            pb = pst[:, 0:2]
            nc.tensor.matmul(pb, ones_row[:], m1n[:], start=True, stop=True)
            nc.vector.tensor_copy(negm[:], pb)
            a_st = stpool.tile([128, NT], F32, tag="ast", name=f"ast{v}")
            b_st = stpool.tile([128, NT], F32, tag="bst", name=f"bst{v}")
            a8 = stpool.tile([128, NT, 1], FP8, tag="a8", name=f"a8{v}")
            nc.scalar.activation(a_st[:], u_all[:], ACTF.Exp, bias=negm[:, 0:1])
            nc.scalar.activation(b_st[:], v_all[:], ACTF.Exp, bias=negm[:, 1:2])
            nc.vector.tensor_scalar(a8[:, :, 0], a_st[:], 1.0, None, op0=MULT)
            s.update(a=a_st, b=b_st, a8=a8)

        def ma_z(v):
            """M @ a via tiny DoubleRow matmuls, one long accum group."""
            s = st[v]
            pa = ppa.tile([128, O], F32, tag="pa", name=f"pa{v}")
            for d in range(NT):
                for j in range(NT // 2):
                    nc.tensor.matmul(
                        pa[:, d : d + 1],
                        mt[:, 2 * j : 2 * j + 2, d * 128 : (d + 1) * 128],
                        s["a8"][:, 2 * j : 2 * j + 2, 0:1],
                        start=(d == 0 and j == 0),
                        stop=(d == NT - 1 and j == NT // 2 - 1),
                        perf_mode=DR,
                        skip_group_check=True,
                    )
            s["pa"] = pa

        def z_chain(v):
            s = st[v]
            zjunk = stpool.tile([128, NT], F32, tag="zj", name=f"zj{v}")
            nc.vector.scalar_tensor_tensor(
                zjunk[:], s["pa"][:, :NT], 1.0, s["b"][:], op0=MULT, op1=MULT,
                accum_out=zp[:, v : v + 1],
            )
            pz = s["pst"][0:1, 4:5]
            nc.tensor.matmul(pz, ones[:], zp[:, v : v + 1], start=True, stop=True)
            nc.vector.reciprocal(z1[0:1, v : v + 1], pz)
            nc.vector.tensor_scalar(
                z1[0:1, v : v + 1], z1[0:1, v : v + 1], 1.0 / V, None, op0=MULT
            )
            przb = s["pst"][:, 8:9]
            nc.tensor.matmul(
                przb, ones_row[:], z1[0:1, v : v + 1], start=True, stop=True
            )
            rzh = stpool.tile([128, 1], F32, tag="rz", name=f"rz{v}")
            nc.vector.tensor_copy(rzh[:], przb)
            brz = stpool.tile([128, NT], F32, tag="brz", name=f"brz{v}")
            nc.vector.tensor_scalar(brz[:], s["b"][:], rzh[:, 0:1], None, op0=MULT)
            s["brz"] = brz

        def q_op(v, t):
            """q[:, t] = a[:, t] * H[:, t] in bf16 (DVE 4x ts)."""
            s = st[v]
            if t % 4 == 0:
                s.setdefault("qgs", {})[t // 4] = gpool.tile(
                    [128, 4, O], BF16, tag="qg", name=f"qg{v}_{t // 4}"
                )
            nc.vector.tensor_scalar(
                s["qgs"][t // 4][:, t % 4, :], s["haug"][:, t, :],
                s["a"][:, t : t + 1], None, op0=MULT,
            )

        def h8_copy(v, t):
            """haug8[:, t] = fp8(q[:, t]) (ACT copy; ACT has slack)."""
            s = st[v]
            nc.scalar.copy(
                s["haug8"][:, t, :], s["qgs"][t // 4][:, t % 4, :]
            )

        def r8_pair(v, p):
            """res8 pair = fp8(q - haug8) for tiles 2p, 2p+1 (Pool/DVE tt)."""
            s = st[v]
            qg = s["qgs"][p // 2]
            k = (2 * p) % 4
            eng = nc.vector if p % 4 == 3 else nc.gpsimd
            eng.tensor_tensor(
                s["res8"][:, 2 * p : 2 * p + 2, :], qg[:, k : k + 2, :],
                s["haug8"][:, 2 * p : 2 * p + 2, :], op=SUB,
            )

        def g_block(v, d):
            """G[:, d] = sum_s M^T[s,d]^T (haug8 + res8)[s]; fp8 DoubleRow."""
            s = st[v]
            pga = ppg.tile([128, O], F32, tag="pg", name=f"pg{v}_{d}")
            nj = NT // 2
            for j in range(nj):
                lhsT = mt[:, 2 * j : 2 * j + 2, d * 128 : (d + 1) * 128]
                nc.tensor.matmul(
                    pga[:], lhsT, s["haug8"][:, 2 * j : 2 * j + 2, :],
                    start=(j == 0), stop=False, perf_mode=DR,
                )
                nc.tensor.matmul(
                    pga[:], lhsT, s["res8"][:, 2 * j : 2 * j + 2, :],
                    start=False, stop=(j == nj - 1), perf_mode=DR,
                )
            tmp = tpool.tile([128, O], F32, tag="tmp", name=f"tmp{v}_{d}")
            nc.scalar.activation(
                tmp[:], pga[:], ACTF.Copy, scale=s["brz"][:, d : d + 1]
            )
            nc.gpsimd.dma_start(out_d[d], tmp[:], accum_op=ADD)

        # ---- emission: G lags its view's elementwise slot by one ----
        view_state(0)
        st[0]["xpc"] = x_loads(0)
        nc.sync.dma_start(attb[:], attb_d[:])
        nc.sync.dma_start(biasb[:], biasb_d[:])
        view_state(1)
        st[1]["xpc"] = x_loads(1)
        for t in range(NT):
            if t == 0:
                for sl in range(NT):
                    nc.sync.dma_start(mt[:, sl, :], mt_d[sl])
                # DRAM out preload with bias (DRAM->DRAM); consumed by the
                # first SWDGE accumulate ~60us in, so emit after the
                # latency-critical input loads
                for d in range(NT):
                    nc.sync.dma_start(out_d[d], biasb_d[:])
            h_tile(0, t)
            if t >= 3:
                dots(0, t - 3)
        for t in range(NT - 3, NT):
            dots(0, t)
        stats(0)

        for sl in range(V + 1):
            cur = sl if sl < V else None        # Ma/z/q for this view
            nxt = sl + 1 if sl + 1 < V else None  # H/dots for this view
            prv = sl - 1 if sl >= 1 else None    # G/P6 for this view
            if nxt is not None and nxt + 1 < V:
                view_state(nxt + 1)
                st[nxt + 1]["xpc"] = x_loads(nxt + 1)
            if nxt is not None:
                view_state(nxt) if nxt not in st else None
            if cur is not None:
                ma_z(cur)
                z_chain(cur)
            for d in range(NT):
                if cur is not None:
                    q_op(cur, d)
                    if d >= 1:
                        h8_copy(cur, d - 1)
                    if d >= 2 and d % 2 == 0:
                        r8_pair(cur, (d - 2) // 2)
                if nxt is not None:
                    h_tile(nxt, d)
                    if d >= 3:
                        dots(nxt, d - 3)
                if prv is not None:
                    g_block(prv, d)
            if cur is not None:
                h8_copy(cur, NT - 1)
                r8_pair(cur, NT // 2 - 2)
                r8_pair(cur, NT // 2 - 1)
            if nxt is not None:
                for t in range(NT - 3, NT):
                    dots(nxt, t)
                stats(nxt)

    nc.compile()
    return nc


_SIGNS = None


def _signs():
    global _SIGNS
    if _SIGNS is None:
        s = np.ones((4, F), dtype=np.float32)
        for r in range(4):
            if r & 1:
                s[r, [0, 2]] = -1.0
            if r & 2:
                s[r, [1, 3]] = -1.0
        _SIGNS = s
    return _SIGNS


def _host_prep(x, edge_index, W, att, bias):
    """Relayout/index preprocessing + exact sign flips + dtype casts +
    tiny W @ att projections for the linear score columns."""
    signs = _signs()
    x = np.ascontiguousarray(x, dtype=np.float32)
    W = np.asarray(W, dtype=np.float32)
    att = np.asarray(att, dtype=np.float32).reshape(2 * O)
    bias = np.asarray(bias, dtype=np.float32)
    ei = np.asarray(edge_index)

    M = np.zeros((N, N), dtype=np.float32)
    np.add.at(M, (ei[1], ei[0]), 1.0)
    M[np.arange(N), np.arange(N)] += 1.0
    MT = np.ascontiguousarray(M.T)
    mt_tiles = np.ascontiguousarray(
        MT.reshape(NT, 128, N).astype(ml_dtypes.float8_e4m3)
    )

    W1, W2 = W[:F], W[F:]
    att_u, att_v = att[:O], att[O:]
    attb04 = np.ascontiguousarray(
        np.broadcast_to((0.8 * att).reshape(1, 2 * O), (128, 2 * O))
    ).astype(np.float32)
    biasb = np.ascontiguousarray(np.broadcast_to(bias, (128, O))).astype(np.float32)

    xT = np.ascontiguousarray(x.transpose(0, 1, 3, 2))  # [B, V, F, N]

    in_maps = []
    for core in range(8):
        b, g = divmod(core, V)
        xpair = np.empty((V, 2, 128, N), dtype=ml_dtypes.bfloat16)
        wselc = np.empty((V, 2, 128, O + 2), dtype=np.float32)
        for h in range(V):
            xpair[h, 0] = xT[b, h]
            xpair[h, 1] = xT[b, g ^ h]
            for i, Wi in ((0, signs[h ^ g][:, None] * W1), (1, signs[h][:, None] * W2)):
                wselc[h, i, :, :O] = Wi
                wselc[h, i, :, O] = Wi @ att_u
                wselc[h, i, :, O + 1] = Wi @ att_v
        in_maps.append(
            {
                "xpair": xpair,
                "wsel": wselc.astype(ml_dtypes.bfloat16),
                "mt": mt_tiles,
                "attb04": attb04,
                "biasb": biasb,
            }
        )
    return in_maps


_NC = None


def kernel(x, edge_index, W, att, bias):
    global _NC
    if _NC is None:
        _NC = _build_program()
    in_maps = _host_prep(x, edge_index, W, att, bias)

    from concourse.bass_utils import run_bass_kernel_spmd

    res = run_bass_kernel_spmd(_NC, in_maps, list(range(8)))
    out = np.empty((B, V, N, O), dtype=np.float32)
    for core in range(8):
        b, g = divmod(core, V)
        out[b, g] = res.results[core]["out"].reshape(N, O)
    return out
